# revision 13
# baseline (speedup 1.0000x reference)
"""Bahdanau attention kernel for 8 Trainium2 NeuronCores.

Math note: in the reference,
    score = (tanh(enc @ Wh + bh) + (dec @ Ws + bs)[:, None, :]) @ Wv + bv
    attn  = softmax(score, axis=T)
the decoder projection term and bv are constant across T, and softmax is
shift-invariant along T — so the decoder branch cancels exactly and both
outputs depend only on softmax_T(tanh(enc @ Wh + bh) @ Wv) and enc itself.
The max-subtraction inside softmax is likewise replaced by a constant
host-computed shift (sum|Wv| + |bh|-slack bounds |score|), so no on-device
max reduction is needed.

Per core (8 batches):
  - big matmul enclinT[u, t] = Wh-tile (stationary) x encT[h, t] (moving) in
    bf16, fp32 PSUM accumulation; tanh (+bh per-partition bias) on ScalarE
  - score[1, t] = Wv-tile (stationary) x tanh tiles, PSUM-accumulated, the
    score matmuls trailing two groups behind the big-MM stream
  - exp on ScalarE with the free-dim accumulator writing the sum into
    column T of the same row; one GpSimdE partition-broadcast of that
    [1, T+1] row feeds both the context multiplies and the 1/sum
  - context in fp32: per h-tile multiply (VectorE) + free-dim reduce
    (VectorE mid-kernel, ScalarE accumulator for the last batch); context
    columns are scaled by 1/sum at the end
Host pre-computes encT (fp32 + bf16) and a j-tiled Wh per core; outputs are
gathered and context^T is rearranged on host.
"""

from contextlib import ExitStack

import numpy as np
import ml_dtypes

import concourse.bass as bass
import concourse.tile as tile
from concourse import bacc, mybir
from concourse.bass_utils import run_bass_kernel_spmd

B, T, H, U = 64, 512, 1024, 1024
NCORES = 8
BL = B // NCORES          # 8 batches per core
P = 128
NH = H // P               # 8 h-tiles
NU = U // P               # 8 u-tiles

f32 = mybir.dt.float32
bf16 = mybir.dt.bfloat16
AF = mybir.ActivationFunctionType
ALU = mybir.AluOpType
BF16 = ml_dtypes.bfloat16


def build_kernel_body(tc, aps, body_ctx):
    nc = tc.nc
    enc_bf_d = aps["enc_bf"]      # [H, BL*T] bf16
    enc_f32_d = aps["enc_f32"]    # [H, BL*T] f32
    wh_d = aps["wh"]              # [NU, H, P] bf16 (j-tiled)
    wv_d = aps["wv"]              # [P, NU] bf16
    bh_d = aps["bh"]              # [P, NU] f32
    nsh_d = aps["nshift"]         # [1, 1] f32: -(bound on score)
    attn_d = aps["attn"]          # [BL, T] f32 out
    ctxt_d = aps["ctxt"]          # [P, NH*BL] f32 out (context^T columns)

    ctx = body_ctx
    cpool = ctx.enter_context(tc.tile_pool(name="const", bufs=1))
    e0_pool = ctx.enter_context(tc.tile_pool(name="e0", bufs=1))
    ebf_pool = ctx.enter_context(tc.tile_pool(name="ebf", bufs=2))
    ef_pool = ctx.enter_context(tc.tile_pool(name="ef", bufs=3))
    th_pool = ctx.enter_context(tc.tile_pool(name="th", bufs=2))
    bc_pool = ctx.enter_context(tc.tile_pool(name="bc", bufs=2))
    scr_pool = ctx.enter_context(tc.tile_pool(name="scr", bufs=3))
    row_pool = ctx.enter_context(tc.tile_pool(name="row", bufs=3))
    ctx_pool = ctx.enter_context(tc.tile_pool(name="ctxt", bufs=1))
    pe_pool = ctx.enter_context(tc.tile_pool(name="pe", bufs=4, space="PSUM"))
    sc_pool = ctx.enter_context(tc.tile_pool(name="sc", bufs=2, space="PSUM"))

    # --- startup-critical loads, all on the sync queue in priority order:
    # j=0 Wh tile, then batch-0 enc bf16 per-h tiles (the first accumulation
    # group can pace with their arrival). Wh j=1..7 go on the scalar queue
    # in per-j tiles so group j only waits for its own 256 KB.
    wh_sb = []
    t_ = cpool.tile([P, H], bf16, tag="wh0")
    nc.sync.dma_start(t_[:].rearrange("p (h c) -> p h c", h=NH),
                      wh_d[0].rearrange("(h p) c -> p h c", p=P))
    wh_sb.append(t_)

    ebf0 = []
    for h in range(NH):
        t_ = e0_pool.tile([P, T], bf16, tag=f"ebf0_{h}")
        nc.sync.dma_start(t_[:], enc_bf_d[h * P:(h + 1) * P, 0:T])
        ebf0.append(t_)

    for j in range(1, NU):
        t_ = cpool.tile([P, H], bf16, tag=f"wh{j}")
        nc.scalar.dma_start(t_[:].rearrange("p (h c) -> p h c", h=NH),
                            wh_d[j].rearrange("(h p) c -> p h c", p=P))
        wh_sb.append(t_)

    wv_sb = cpool.tile([P, NU], bf16, tag="wv")
    nc.sync.dma_start(wv_sb[:], wv_d[:, :])
    bh_sb = cpool.tile([P, NU], f32, tag="bh")
    nc.sync.dma_start(bh_sb[:], bh_d[:, :])
    nsh_sb = cpool.tile([1, 1], f32, tag="nsh")
    nc.sync.dma_start(nsh_sb[:], nsh_d[:, :])

    ctx_all = ctx_pool.tile([P, NH * BL], f32, tag="ctxall")
    ctx_view = ctx_all[:].rearrange("p (h b) -> p h b", b=BL)

    # one DMA per batch for the streaming enc tiles
    def load_ebf(b):
        t_ = ebf_pool.tile([P, NH * T], bf16, tag="ebfall")
        nc.sync.dma_start(
            t_[:].rearrange("p (h t) -> p h t", h=NH),
            enc_bf_d.rearrange("(h p) t -> p h t", p=P)[:, :,
                                                        b * T:(b + 1) * T])
        return t_

    def load_ef(b):
        t_ = ef_pool.tile([P, NH * T], f32, tag="efall")
        nc.sync.dma_start(
            t_[:].rearrange("p (h t) -> p h t", h=NH),
            enc_f32_d.rearrange("(h p) t -> p h t", p=P)[:, :,
                                                         b * T:(b + 1) * T])
        return t_

    ef_cur = load_ef(0)
    ebf_cur = None

    def issue_softmax_and_context(b, ef, score_ps, last):
        """Everything after batch b's score matmuls."""
        # exp row with the free-dim sum accumulated into column T
        erow = row_pool.tile([1, T + 1], f32, tag="erow")
        nc.scalar.activation(erow[:, 0:T], score_ps[:], AF.Exp,
                             bias=nsh_sb[:], accum_out=erow[:, T:T + 1])
        # one broadcast feeds the context multiplies AND the 1/sum
        bc = bc_pool.tile([P, T + 1], f32, tag="bc")
        nc.gpsimd.partition_broadcast(bc[:], erow[:])
        rbc = row_pool.tile([P, 1], f32, tag="rbc")
        nc.vector.reciprocal(rbc[:], bc[:, T:T + 1])
        arow = row_pool.tile([1, T], f32, tag="arow")
        nc.vector.tensor_scalar_mul(arow[:], erow[:, 0:T], rbc[0:1, 0:1])
        nc.sync.dma_start(attn_d[b:b + 1, :], arow[:])

        for h in range(NH):
            scr = scr_pool.tile([P, T], f32, tag="scr")
            nc.vector.tensor_mul(scr[:], ef[:, h * T:(h + 1) * T], bc[:, 0:T])
            col = ctx_view[:, h, b:b + 1]
            if last:
                # keep the exposed tail short: reduce on ScalarE via the
                # activation accumulator while VectorE keeps multiplying
                scr2 = scr_pool.tile([P, T], f32, tag="scr2")
                nc.scalar.activation(scr2[:], scr[:], AF.Identity, bias=0.0,
                                     accum_out=col)
            else:
                nc.vector.tensor_reduce(col, scr[:],
                                        axis=mybir.AxisListType.X, op=ALU.add)
        # normalize this batch's context columns by 1/sum
        nc.vector.tensor_scalar_mul(ctx_view[:, :, b], ctx_view[:, :, b],
                                    rbc[:])

    for b in range(BL):
        ef = ef_cur
        ebf = ebf_cur
        tanh_ts = []
        score_ps = sc_pool.tile([1, T], f32, tag="score")

        def score_mm(j):
            nc.tensor.matmul(
                score_ps[:], wv_sb[:, j:j + 1], tanh_ts[j][:],
                start=(j == 0), stop=(j == NU - 1), skip_group_check=True,
            )

        for j in range(NU):
            pe = pe_pool.tile([P, T], f32, tag="pe")
            for h in range(NH):
                rhs = (ebf0[h][:] if b == 0
                       else ebf[:, h * T:(h + 1) * T])
                nc.tensor.matmul(
                    pe[:], wh_sb[j][:, h * P:(h + 1) * P], rhs,
                    start=(h == 0), stop=(h == NH - 1),
                )
            th = th_pool.tile([P, T], bf16, tag=f"th{j}")
            nc.scalar.activation(th[:], pe[:], AF.Tanh, bias=bh_sb[:, j:j + 1])
            tanh_ts.append(th)
            # score matmuls trail the big-MM stream by two groups so their
            # tanh inputs are always ready when the PE reaches them
            if j >= 2:
                score_mm(j - 2)
            if j == 1 and b + 1 < BL:
                ebf_cur = load_ebf(b + 1)
                ef_cur = load_ef(b + 1)
        score_mm(NU - 2)
        score_mm(NU - 1)
        issue_softmax_and_context(b, ef, score_ps, last=(b == BL - 1))

    nc.sync.dma_start(ctxt_d[:, :], ctx_all[:])


def build_nc():
    nc = bacc.Bacc("TRN2", target_bir_lowering=False, debug=False,
                   num_devices=NCORES)
    aps = {
        "enc_bf": nc.dram_tensor("enc_bf", [H, BL * T], bf16,
                                 kind="ExternalInput").ap(),
        "enc_f32": nc.dram_tensor("enc_f32", [H, BL * T], f32,
                                  kind="ExternalInput").ap(),
        "wh": nc.dram_tensor("wh", [NU, H, P], bf16,
                             kind="ExternalInput").ap(),
        "wv": nc.dram_tensor("wv", [P, NU], bf16, kind="ExternalInput").ap(),
        "bh": nc.dram_tensor("bh", [P, NU], f32, kind="ExternalInput").ap(),
        "nshift": nc.dram_tensor("nshift", [1, 1], f32,
                                 kind="ExternalInput").ap(),
        "attn": nc.dram_tensor("attn", [BL, T], f32,
                               kind="ExternalOutput").ap(),
        "ctxt": nc.dram_tensor("ctxt", [P, NH * BL], f32,
                               kind="ExternalOutput").ap(),
    }
    with tile.TileContext(nc) as tc:
        with ExitStack() as body_ctx:
            build_kernel_body(tc, aps, body_ctx)
    nc.compile()
    return nc


def make_in_maps(enc_output, Wh, bh, Wv):
    enc = np.ascontiguousarray(np.asarray(enc_output, dtype=np.float32))
    wh = np.asarray(Wh, dtype=np.float32)
    # j-tiled Wh: wh_tiled[j, h*P+p, c] = Wh[h*P+p, j*P+c]
    wh_tiled = np.ascontiguousarray(
        wh.reshape(H, NU, P).transpose(1, 0, 2)).astype(BF16)
    wv = np.asarray(Wv, dtype=np.float32)
    wv_t = np.ascontiguousarray(wv.reshape(NU, P).T).astype(BF16)
    bh_t = np.ascontiguousarray(
        np.asarray(bh, dtype=np.float32).reshape(NU, P).T)
    # |score| <= sum_u |Wv_u| * |tanh| <= sum|Wv|; constant softmax shift
    # (softmax is shift-invariant, exp(+-bound) stays well inside fp32)
    nshift = np.array([[-min(float(np.abs(wv).sum()), 60.0)]],
                      dtype=np.float32)
    in_maps = []
    for c in range(NCORES):
        shard = enc[c * BL:(c + 1) * BL].reshape(BL * T, H)
        encT = np.ascontiguousarray(shard.T)          # [H, BL*T] f32
        in_maps.append({
            "enc_bf": encT.astype(BF16),
            "enc_f32": encT,
            "wh": wh_tiled,
            "wv": wv_t,
            "bh": bh_t,
            "nshift": nshift,
        })
    return in_maps


_NC_CACHE = None


def kernel(dec_hidden, enc_output, Wh, bh, Ws, bs, Wv, bv, **_unused):
    global _NC_CACHE
    if _NC_CACHE is None:
        _NC_CACHE = build_nc()
    nc = _NC_CACHE
    in_maps = make_in_maps(enc_output, Wh, bh, Wv)
    res = run_bass_kernel_spmd(nc, in_maps, list(range(NCORES))).results
    attn = np.concatenate([res[c]["attn"] for c in range(NCORES)], axis=0)
    ctx_parts = []
    for c in range(NCORES):
        ct = res[c]["ctxt"]                           # [P, NH*BL]
        ct = ct.reshape(P, NH, BL).transpose(2, 1, 0).reshape(BL, H)
        ctx_parts.append(np.ascontiguousarray(ct))
    context = np.concatenate(ctx_parts, axis=0)
    return context.astype(np.float32), attn.astype(np.float32)


# revision 14
# speedup vs baseline: 1.0958x; 1.0958x over previous
"""Bahdanau attention kernel for 8 Trainium2 NeuronCores.

Math note: in the reference,
    score = (tanh(enc @ Wh + bh) + (dec @ Ws + bs)[:, None, :]) @ Wv + bv
    attn  = softmax(score, axis=T)
the decoder projection term and bv are constant across T, and softmax is
shift-invariant along T — so the decoder branch cancels exactly and both
outputs depend only on softmax_T(tanh(enc @ Wh + bh) @ Wv) and enc itself.
The max-subtraction inside softmax is likewise replaced by a constant
host-computed shift (sum|Wv| + |bh|-slack bounds |score|), so no on-device
max reduction is needed.

Per core (8 batches):
  - big matmul enclinT[u, t] = Wh-tile (stationary) x encT[h, t] (moving) in
    bf16, fp32 PSUM accumulation; tanh (+bh per-partition bias) on ScalarE
  - score[1, t] = Wv-tile (stationary) x tanh tiles, PSUM-accumulated, the
    score matmuls trailing two groups behind the big-MM stream
  - exp on ScalarE with the free-dim accumulator writing the sum into
    column T of the same row; one GpSimdE partition-broadcast of that
    [1, T+1] row feeds both the context multiplies and the 1/sum
  - context in fp32: per h-tile multiply (VectorE) + free-dim reduce
    (VectorE mid-kernel, ScalarE accumulator for the last batch); context
    columns are scaled by 1/sum at the end
Host pre-computes encT (fp32 + bf16) and a j-tiled Wh per core; outputs are
gathered and context^T is rearranged on host.
"""

from contextlib import ExitStack

import numpy as np
import ml_dtypes

import concourse.bass as bass
import concourse.tile as tile
from concourse import bacc, mybir
from concourse.bass_utils import run_bass_kernel_spmd

B, T, H, U = 64, 512, 1024, 1024
NCORES = 8
BL = B // NCORES          # 8 batches per core
P = 128
NH = H // P               # 8 h-tiles
NU = U // P               # 8 u-tiles

f32 = mybir.dt.float32
bf16 = mybir.dt.bfloat16
AF = mybir.ActivationFunctionType
ALU = mybir.AluOpType
BF16 = ml_dtypes.bfloat16


def build_kernel_body(tc, aps, body_ctx):
    nc = tc.nc
    enc_bf_d = aps["enc_bf"]      # [H, BL*T] bf16
    enc_f32_d = aps["enc_f32"]    # [H, BL*T] f32
    wh_d = aps["wh"]              # [NU, H, P] bf16 (j-tiled)
    wv_d = aps["wv"]              # [P, NU] bf16
    bh_d = aps["bh"]              # [P, NU] f32
    nsh_d = aps["nshift"]         # [1, 1] f32: -(bound on score)
    attn_d = aps["attn"]          # [BL, T] f32 out
    ctxt_d = aps["ctxt"]          # [P, NH*BL] f32 out (context^T columns)

    ctx = body_ctx
    cpool = ctx.enter_context(tc.tile_pool(name="const", bufs=1))
    e0_pool = ctx.enter_context(tc.tile_pool(name="e0", bufs=1))
    ebf_pool = ctx.enter_context(tc.tile_pool(name="ebf", bufs=2))
    ef_pool = ctx.enter_context(tc.tile_pool(name="ef", bufs=3))
    th_pool = ctx.enter_context(tc.tile_pool(name="th", bufs=2))
    bc_pool = ctx.enter_context(tc.tile_pool(name="bc", bufs=2))
    scr_pool = ctx.enter_context(tc.tile_pool(name="scr", bufs=3))
    row_pool = ctx.enter_context(tc.tile_pool(name="row", bufs=3))
    ctx_pool = ctx.enter_context(tc.tile_pool(name="ctxt", bufs=1))
    pe_pool = ctx.enter_context(tc.tile_pool(name="pe", bufs=6, space="PSUM"))
    sc_pool = ctx.enter_context(tc.tile_pool(name="sc", bufs=2, space="PSUM"))

    # --- startup-critical loads, all on the sync queue in priority order:
    # j=0 Wh tile, then batch-0 enc bf16 per-h tiles (the first accumulation
    # group can pace with their arrival). Wh j=1..7 go on the scalar queue
    # in per-j tiles so group j only waits for its own 256 KB.
    wh_sb = []
    t_ = cpool.tile([P, H], bf16, tag="wh0")
    nc.sync.dma_start(t_[:].rearrange("p (h c) -> p h c", h=NH),
                      wh_d[0].rearrange("(h p) c -> p h c", p=P))
    wh_sb.append(t_)

    ebf0 = []
    for h in range(NH):
        t_ = e0_pool.tile([P, T], bf16, tag=f"ebf0_{h}")
        eng = nc.sync if h < 4 else nc.scalar
        eng.dma_start(t_[:], enc_bf_d[h * P:(h + 1) * P, 0:T])
        ebf0.append(t_)

    for j in range(1, NU):
        t_ = cpool.tile([P, H], bf16, tag=f"wh{j}")
        eng = nc.sync if j % 2 == 0 else nc.scalar
        eng.dma_start(t_[:].rearrange("p (h c) -> p h c", h=NH),
                      wh_d[j].rearrange("(h p) c -> p h c", p=P))
        wh_sb.append(t_)

    wv_sb = cpool.tile([P, NU], bf16, tag="wv")
    nc.sync.dma_start(wv_sb[:], wv_d[:, :])
    bh_sb = cpool.tile([P, NU], f32, tag="bh")
    nc.sync.dma_start(bh_sb[:], bh_d[:, :])
    nsh_sb = cpool.tile([1, 1], f32, tag="nsh")
    nc.sync.dma_start(nsh_sb[:], nsh_d[:, :])

    ctx_all = ctx_pool.tile([P, NH * BL], f32, tag="ctxall")
    ctx_view = ctx_all[:].rearrange("p (h b) -> p h b", b=BL)

    # one DMA per batch for the streaming enc tiles
    def load_ebf(b):
        t_ = ebf_pool.tile([P, NH * T], bf16, tag="ebfall")
        nc.sync.dma_start(
            t_[:].rearrange("p (h t) -> p h t", h=NH),
            enc_bf_d.rearrange("(h p) t -> p h t", p=P)[:, :,
                                                        b * T:(b + 1) * T])
        return t_

    def load_ef(b):
        t_ = ef_pool.tile([P, NH * T], f32, tag="efall")
        nc.sync.dma_start(
            t_[:].rearrange("p (h t) -> p h t", h=NH),
            enc_f32_d.rearrange("(h p) t -> p h t", p=P)[:, :,
                                                         b * T:(b + 1) * T])
        return t_

    ef_cur = load_ef(0)
    ebf_cur = None

    def issue_softmax_and_context(b, ef, score_ps, last):
        """Everything after batch b's score matmuls."""
        # exp row with the free-dim sum accumulated into column T
        erow = row_pool.tile([1, T + 1], f32, tag="erow")
        nc.scalar.activation(erow[:, 0:T], score_ps[:], AF.Exp,
                             bias=nsh_sb[:], accum_out=erow[:, T:T + 1])
        # one broadcast feeds the context multiplies AND the 1/sum
        bc = bc_pool.tile([P, T + 1], f32, tag="bc")
        nc.gpsimd.partition_broadcast(bc[:], erow[:])
        rbc = row_pool.tile([P, 1], f32, tag="rbc")
        nc.vector.reciprocal(rbc[:], bc[:, T:T + 1])
        arow = row_pool.tile([1, T], f32, tag="arow")
        nc.vector.tensor_scalar_mul(arow[:], erow[:, 0:T], rbc[0:1, 0:1])
        nc.sync.dma_start(attn_d[b:b + 1, :], arow[:])

        for h in range(NH):
            scr = scr_pool.tile([P, T], f32, tag="scr")
            nc.vector.tensor_mul(scr[:], ef[:, h * T:(h + 1) * T], bc[:, 0:T])
            col = ctx_view[:, h, b:b + 1]
            if last:
                # keep the exposed tail short: reduce on ScalarE via the
                # activation accumulator while VectorE keeps multiplying
                scr2 = scr_pool.tile([P, T], f32, tag="scr2")
                nc.scalar.activation(scr2[:], scr[:], AF.Identity, bias=0.0,
                                     accum_out=col)
            else:
                nc.vector.tensor_reduce(col, scr[:],
                                        axis=mybir.AxisListType.X, op=ALU.add)
        # normalize this batch's context columns by 1/sum
        nc.vector.tensor_scalar_mul(ctx_view[:, :, b], ctx_view[:, :, b],
                                    rbc[:])

    def make_score_mm(tanh_ts, score_ps):
        def score_mm(j):
            nc.tensor.matmul(
                score_ps[:], wv_sb[:, j:j + 1], tanh_ts[j][:],
                start=(j == 0), stop=(j == NU - 1), skip_group_check=True,
            )
        return score_mm

    prev = None  # (b-1's tanh tiles, ef tile, score psum)
    for b in range(BL):
        ef = ef_cur
        ebf = ebf_cur
        last = b == BL - 1
        tanh_ts = []
        score_ps = sc_pool.tile([1, T], f32, tag="score")
        score_mm = make_score_mm(tanh_ts, score_ps)
        for j in range(NU):
            pe = pe_pool.tile([P, T], f32, tag="pe")
            for h in range(NH):
                rhs = (ebf0[h][:] if b == 0
                       else ebf[:, h * T:(h + 1) * T])
                nc.tensor.matmul(
                    pe[:], wh_sb[j][:, h * P:(h + 1) * P], rhs,
                    start=(h == 0), stop=(h == NH - 1),
                )
            th = th_pool.tile([P, T], bf16, tag=f"th{j}")
            nc.scalar.activation(th[:], pe[:], AF.Tanh, bias=bh_sb[:, j:j + 1])
            tanh_ts.append(th)
            if j == 0 and prev is not None:
                # previous batch's score matmuls + softmax + context slot in
                # here: tanh(b, 0) is issued first so ScalarE's FIFO reaches
                # it before the exp
                pt, pef, pps = prev
                psc = make_score_mm(pt, pps)
                for jj in range(NU):
                    psc(jj)
                issue_softmax_and_context(b - 1, pef, pps, last=False)
            if last and j >= 2:
                # last batch: scores trail two groups behind so the final
                # block of score matmuls is short
                score_mm(j - 2)
            if j == 1 and b + 1 < BL:
                ebf_cur = load_ebf(b + 1)
                ef_cur = load_ef(b + 1)
        if last:
            score_mm(NU - 2)
            score_mm(NU - 1)
            issue_softmax_and_context(b, ef, score_ps, last=True)
        else:
            prev = (tanh_ts, ef, score_ps)

    nc.sync.dma_start(ctxt_d[:, :], ctx_all[:])


def build_nc():
    nc = bacc.Bacc("TRN2", target_bir_lowering=False, debug=False,
                   num_devices=NCORES)
    aps = {
        "enc_bf": nc.dram_tensor("enc_bf", [H, BL * T], bf16,
                                 kind="ExternalInput").ap(),
        "enc_f32": nc.dram_tensor("enc_f32", [H, BL * T], f32,
                                  kind="ExternalInput").ap(),
        "wh": nc.dram_tensor("wh", [NU, H, P], bf16,
                             kind="ExternalInput").ap(),
        "wv": nc.dram_tensor("wv", [P, NU], bf16, kind="ExternalInput").ap(),
        "bh": nc.dram_tensor("bh", [P, NU], f32, kind="ExternalInput").ap(),
        "nshift": nc.dram_tensor("nshift", [1, 1], f32,
                                 kind="ExternalInput").ap(),
        "attn": nc.dram_tensor("attn", [BL, T], f32,
                               kind="ExternalOutput").ap(),
        "ctxt": nc.dram_tensor("ctxt", [P, NH * BL], f32,
                               kind="ExternalOutput").ap(),
    }
    with tile.TileContext(nc) as tc:
        with ExitStack() as body_ctx:
            build_kernel_body(tc, aps, body_ctx)
    nc.compile()
    return nc


def make_in_maps(enc_output, Wh, bh, Wv):
    enc = np.ascontiguousarray(np.asarray(enc_output, dtype=np.float32))
    wh = np.asarray(Wh, dtype=np.float32)
    # j-tiled Wh: wh_tiled[j, h*P+p, c] = Wh[h*P+p, j*P+c]
    wh_tiled = np.ascontiguousarray(
        wh.reshape(H, NU, P).transpose(1, 0, 2)).astype(BF16)
    wv = np.asarray(Wv, dtype=np.float32)
    wv_t = np.ascontiguousarray(wv.reshape(NU, P).T).astype(BF16)
    bh_t = np.ascontiguousarray(
        np.asarray(bh, dtype=np.float32).reshape(NU, P).T)
    # |score| <= sum_u |Wv_u| * |tanh| <= sum|Wv|; constant softmax shift
    # (softmax is shift-invariant, exp(+-bound) stays well inside fp32)
    nshift = np.array([[-min(float(np.abs(wv).sum()), 60.0)]],
                      dtype=np.float32)
    in_maps = []
    for c in range(NCORES):
        shard = enc[c * BL:(c + 1) * BL].reshape(BL * T, H)
        encT = np.ascontiguousarray(shard.T)          # [H, BL*T] f32
        in_maps.append({
            "enc_bf": encT.astype(BF16),
            "enc_f32": encT,
            "wh": wh_tiled,
            "wv": wv_t,
            "bh": bh_t,
            "nshift": nshift,
        })
    return in_maps


_NC_CACHE = None


def kernel(dec_hidden, enc_output, Wh, bh, Ws, bs, Wv, bv, **_unused):
    global _NC_CACHE
    if _NC_CACHE is None:
        _NC_CACHE = build_nc()
    nc = _NC_CACHE
    in_maps = make_in_maps(enc_output, Wh, bh, Wv)
    res = run_bass_kernel_spmd(nc, in_maps, list(range(NCORES))).results
    attn = np.concatenate([res[c]["attn"] for c in range(NCORES)], axis=0)
    ctx_parts = []
    for c in range(NCORES):
        ct = res[c]["ctxt"]                           # [P, NH*BL]
        ct = ct.reshape(P, NH, BL).transpose(2, 1, 0).reshape(BL, H)
        ctx_parts.append(np.ascontiguousarray(ct))
    context = np.concatenate(ctx_parts, axis=0)
    return context.astype(np.float32), attn.astype(np.float32)


# revision 16
# speedup vs baseline: 1.1047x; 1.0082x over previous
"""Bahdanau attention kernel for 8 Trainium2 NeuronCores.

Math note: in the reference,
    score = (tanh(enc @ Wh + bh) + (dec @ Ws + bs)[:, None, :]) @ Wv + bv
    attn  = softmax(score, axis=T)
the decoder projection term and bv are constant across T, and softmax is
shift-invariant along T — so the decoder branch cancels exactly and both
outputs depend only on softmax_T(tanh(enc @ Wh + bh) @ Wv) and enc itself.
The max-subtraction inside softmax is likewise replaced by a constant
host-computed shift (sum|Wv| + |bh|-slack bounds |score|), so no on-device
max reduction is needed.

Per core (8 batches):
  - big matmul enclinT[u, t] = Wh-tile (stationary) x encT[h, t] (moving) in
    bf16, fp32 PSUM accumulation; tanh (+bh per-partition bias) on ScalarE
  - score[1, t] = Wv-tile (stationary) x tanh tiles, PSUM-accumulated, the
    score matmuls trailing two groups behind the big-MM stream
  - exp on ScalarE with the free-dim accumulator writing the sum into
    column T of the same row; one GpSimdE partition-broadcast of that
    [1, T+1] row feeds both the context multiplies and the 1/sum
  - context in fp32: per h-tile multiply (VectorE) + free-dim reduce
    (VectorE mid-kernel, ScalarE accumulator for the last batch); context
    columns are scaled by 1/sum at the end
Host pre-computes encT (fp32 + bf16) and a j-tiled Wh per core; outputs are
gathered and context^T is rearranged on host.
"""

from contextlib import ExitStack

import numpy as np
import ml_dtypes

import concourse.bass as bass
import concourse.tile as tile
from concourse import bacc, mybir
from concourse.bass_utils import run_bass_kernel_spmd

B, T, H, U = 64, 512, 1024, 1024
NCORES = 8
BL = B // NCORES          # 8 batches per core
P = 128
NH = H // P               # 8 h-tiles
NU = U // P               # 8 u-tiles

f32 = mybir.dt.float32
bf16 = mybir.dt.bfloat16
AF = mybir.ActivationFunctionType
ALU = mybir.AluOpType
BF16 = ml_dtypes.bfloat16


def build_kernel_body(tc, aps, body_ctx):
    nc = tc.nc
    enc_bf_d = aps["enc_bf"]      # [H, BL*T] bf16
    enc_f32_d = aps["enc_f32"]    # [H, BL*T] f32
    wh_d = aps["wh"]              # [NU, H, P] bf16 (j-tiled)
    wv_d = aps["wv"]              # [P, NU] bf16
    bh_d = aps["bh"]              # [P, NU] f32
    nsh_d = aps["nshift"]         # [1, 1] f32: -(bound on score)
    attn_d = aps["attn"]          # [BL, T] f32 out
    ctxt_d = aps["ctxt"]          # [P, NH*BL] f32 out (context^T columns)

    ctx = body_ctx
    cpool = ctx.enter_context(tc.tile_pool(name="const", bufs=1))
    e0_pool = ctx.enter_context(tc.tile_pool(name="e0", bufs=1))
    ebf_pool = ctx.enter_context(tc.tile_pool(name="ebf", bufs=2))
    ef_pool = ctx.enter_context(tc.tile_pool(name="ef", bufs=3))
    th_pool = ctx.enter_context(tc.tile_pool(name="th", bufs=3))
    bc_pool = ctx.enter_context(tc.tile_pool(name="bc", bufs=2))
    scr_pool = ctx.enter_context(tc.tile_pool(name="scr", bufs=3))
    row_pool = ctx.enter_context(tc.tile_pool(name="row", bufs=3))
    ctx_pool = ctx.enter_context(tc.tile_pool(name="ctxt", bufs=1))
    pe_pool = ctx.enter_context(tc.tile_pool(name="pe", bufs=6, space="PSUM"))
    sc_pool = ctx.enter_context(tc.tile_pool(name="sc", bufs=2, space="PSUM"))

    # --- startup-critical loads, all on the sync queue in priority order:
    # j=0 Wh tile, then batch-0 enc bf16 per-h tiles (the first accumulation
    # group can pace with their arrival). Wh j=1..7 go on the scalar queue
    # in per-j tiles so group j only waits for its own 256 KB.
    wh_sb = []
    t_ = cpool.tile([P, H], bf16, tag="wh0")
    nc.sync.dma_start(t_[:].rearrange("p (h c) -> p h c", h=NH),
                      wh_d[0].rearrange("(h p) c -> p h c", p=P))
    wh_sb.append(t_)

    ebf0 = []
    for h in range(NH):
        t_ = e0_pool.tile([P, T], bf16, tag=f"ebf0_{h}")
        eng = nc.sync if h < 4 else nc.scalar
        eng.dma_start(t_[:], enc_bf_d[h * P:(h + 1) * P, 0:T])
        ebf0.append(t_)

    for j in range(1, NU):
        t_ = cpool.tile([P, H], bf16, tag=f"wh{j}")
        eng = nc.sync if j % 2 == 0 else nc.scalar
        eng.dma_start(t_[:].rearrange("p (h c) -> p h c", h=NH),
                      wh_d[j].rearrange("(h p) c -> p h c", p=P))
        wh_sb.append(t_)

    wv_sb = cpool.tile([P, NU], bf16, tag="wv")
    nc.scalar.dma_start(wv_sb[:], wv_d[:, :])
    bh_sb = cpool.tile([P, NU], f32, tag="bh")
    nc.scalar.dma_start(bh_sb[:], bh_d[:, :])
    nsh_sb = cpool.tile([1, 1], f32, tag="nsh")
    nc.scalar.dma_start(nsh_sb[:], nsh_d[:, :])

    ctx_all = ctx_pool.tile([P, NH * BL], f32, tag="ctxall")
    ctx_view = ctx_all[:].rearrange("p (h b) -> p h b", b=BL)

    # one DMA per batch for the streaming enc tiles
    def load_ebf(b):
        t_ = ebf_pool.tile([P, NH * T], bf16, tag="ebfall")
        nc.sync.dma_start(
            t_[:].rearrange("p (h t) -> p h t", h=NH),
            enc_bf_d.rearrange("(h p) t -> p h t", p=P)[:, :,
                                                        b * T:(b + 1) * T])
        return t_

    def load_ef(b):
        t_ = ef_pool.tile([P, NH * T], f32, tag="efall")
        nc.sync.dma_start(
            t_[:].rearrange("p (h t) -> p h t", h=NH),
            enc_f32_d.rearrange("(h p) t -> p h t", p=P)[:, :,
                                                         b * T:(b + 1) * T])
        return t_

    ef_cur = None
    ebf_cur = None

    def issue_softmax_and_context(b, ef, score_ps, last):
        """Everything after batch b's score matmuls."""
        # exp row with the free-dim sum accumulated into column T
        erow = row_pool.tile([1, T + 1], f32, tag="erow")
        nc.scalar.activation(erow[:, 0:T], score_ps[:], AF.Exp,
                             bias=nsh_sb[:], accum_out=erow[:, T:T + 1])
        # one broadcast feeds the context multiplies AND the 1/sum
        bc = bc_pool.tile([P, T + 1], f32, tag="bc")
        nc.gpsimd.partition_broadcast(bc[:], erow[:])
        rbc = row_pool.tile([P, 1], f32, tag="rbc")
        nc.vector.reciprocal(rbc[:], bc[:, T:T + 1])
        arow = row_pool.tile([1, T], f32, tag="arow")
        nc.vector.tensor_scalar_mul(arow[:], erow[:, 0:T], rbc[0:1, 0:1])
        nc.sync.dma_start(attn_d[b:b + 1, :], arow[:])

        for h in range(NH):
            # one VectorE op per h-tile: (enc * 1/sum) * exp_bc, with the
            # free-dim accumulator writing the normalized context column
            scr = scr_pool.tile([P, T], f32, tag="scr")
            nc.vector.scalar_tensor_tensor(
                out=scr[:], in0=ef[:, h * T:(h + 1) * T], scalar=rbc[:],
                in1=bc[:, 0:T], op0=ALU.mult, op1=ALU.mult,
                accum_out=ctx_view[:, h, b:b + 1])

    def make_score_mm(tanh_ts, score_ps):
        def score_mm(j):
            nc.tensor.matmul(
                score_ps[:], wv_sb[:, j:j + 1], tanh_ts[j][:],
                start=(j == 0), stop=(j == NU - 1), skip_group_check=True,
            )
        return score_mm

    prev = None  # (b-1's tanh tiles, ef tile, score psum)
    for b in range(BL):
        ef = ef_cur
        ebf = ebf_cur
        last = b == BL - 1
        tanh_ts = []
        score_ps = sc_pool.tile([1, T], f32, tag="score")
        score_mm = make_score_mm(tanh_ts, score_ps)
        for j in range(NU):
            pe = pe_pool.tile([P, T], f32, tag="pe")
            for h in range(NH):
                rhs = (ebf0[h][:] if b == 0
                       else ebf[:, h * T:(h + 1) * T])
                nc.tensor.matmul(
                    pe[:], wh_sb[j][:, h * P:(h + 1) * P], rhs,
                    start=(h == 0), stop=(h == NH - 1),
                )
            th = th_pool.tile([P, T], bf16, tag=f"th{j}")
            nc.scalar.activation(th[:], pe[:], AF.Tanh, bias=bh_sb[:, j:j + 1])
            tanh_ts.append(th)
            if j == 0 and prev is not None:
                # previous batch's score matmuls + softmax + context slot in
                # here: tanh(b, 0) is issued first so ScalarE's FIFO reaches
                # it before the exp
                pt, pef, pps = prev
                psc = make_score_mm(pt, pps)
                for jj in range(NU):
                    psc(jj)
                issue_softmax_and_context(b - 1, pef, pps, last=False)
            if last and j >= 2:
                # last batch: scores trail two groups behind so the final
                # block of score matmuls is short
                score_mm(j - 2)
            if j == 1 and b + 1 < BL:
                ebf_cur = load_ebf(b + 1)
                ef_cur = load_ef(b + 1)
            if j == 2 and b == 0:
                ef = load_ef(0)
        if last:
            score_mm(NU - 2)
            score_mm(NU - 1)
            issue_softmax_and_context(b, ef, score_ps, last=True)
        else:
            prev = (tanh_ts, ef, score_ps)

    nc.sync.dma_start(ctxt_d[:, :], ctx_all[:])


def build_nc():
    nc = bacc.Bacc("TRN2", target_bir_lowering=False, debug=False,
                   num_devices=NCORES)
    aps = {
        "enc_bf": nc.dram_tensor("enc_bf", [H, BL * T], bf16,
                                 kind="ExternalInput").ap(),
        "enc_f32": nc.dram_tensor("enc_f32", [H, BL * T], f32,
                                  kind="ExternalInput").ap(),
        "wh": nc.dram_tensor("wh", [NU, H, P], bf16,
                             kind="ExternalInput").ap(),
        "wv": nc.dram_tensor("wv", [P, NU], bf16, kind="ExternalInput").ap(),
        "bh": nc.dram_tensor("bh", [P, NU], f32, kind="ExternalInput").ap(),
        "nshift": nc.dram_tensor("nshift", [1, 1], f32,
                                 kind="ExternalInput").ap(),
        "attn": nc.dram_tensor("attn", [BL, T], f32,
                               kind="ExternalOutput").ap(),
        "ctxt": nc.dram_tensor("ctxt", [P, NH * BL], f32,
                               kind="ExternalOutput").ap(),
    }
    with tile.TileContext(nc) as tc:
        with ExitStack() as body_ctx:
            build_kernel_body(tc, aps, body_ctx)
    nc.compile()
    return nc


def make_in_maps(enc_output, Wh, bh, Wv):
    enc = np.ascontiguousarray(np.asarray(enc_output, dtype=np.float32))
    wh = np.asarray(Wh, dtype=np.float32)
    # j-tiled Wh: wh_tiled[j, h*P+p, c] = Wh[h*P+p, j*P+c]
    wh_tiled = np.ascontiguousarray(
        wh.reshape(H, NU, P).transpose(1, 0, 2)).astype(BF16)
    wv = np.asarray(Wv, dtype=np.float32)
    wv_t = np.ascontiguousarray(wv.reshape(NU, P).T).astype(BF16)
    bh_t = np.ascontiguousarray(
        np.asarray(bh, dtype=np.float32).reshape(NU, P).T)
    # |score| <= sum_u |Wv_u| * |tanh| <= sum|Wv|; constant softmax shift
    # (softmax is shift-invariant, exp(+-bound) stays well inside fp32)
    nshift = np.array([[-min(float(np.abs(wv).sum()), 60.0)]],
                      dtype=np.float32)
    in_maps = []
    for c in range(NCORES):
        shard = enc[c * BL:(c + 1) * BL].reshape(BL * T, H)
        encT = np.ascontiguousarray(shard.T)          # [H, BL*T] f32
        in_maps.append({
            "enc_bf": encT.astype(BF16),
            "enc_f32": encT,
            "wh": wh_tiled,
            "wv": wv_t,
            "bh": bh_t,
            "nshift": nshift,
        })
    return in_maps


_NC_CACHE = None


def kernel(dec_hidden, enc_output, Wh, bh, Ws, bs, Wv, bv, **_unused):
    global _NC_CACHE
    if _NC_CACHE is None:
        _NC_CACHE = build_nc()
    nc = _NC_CACHE
    in_maps = make_in_maps(enc_output, Wh, bh, Wv)
    res = run_bass_kernel_spmd(nc, in_maps, list(range(NCORES))).results
    attn = np.concatenate([res[c]["attn"] for c in range(NCORES)], axis=0)
    ctx_parts = []
    for c in range(NCORES):
        ct = res[c]["ctxt"]                           # [P, NH*BL]
        ct = ct.reshape(P, NH, BL).transpose(2, 1, 0).reshape(BL, H)
        ctx_parts.append(np.ascontiguousarray(ct))
    context = np.concatenate(ctx_parts, axis=0)
    return context.astype(np.float32), attn.astype(np.float32)


# revision 17
# speedup vs baseline: 1.1131x; 1.0076x over previous
"""Bahdanau attention kernel for 8 Trainium2 NeuronCores.

Math note: in the reference,
    score = (tanh(enc @ Wh + bh) + (dec @ Ws + bs)[:, None, :]) @ Wv + bv
    attn  = softmax(score, axis=T)
the decoder projection term and bv are constant across T, and softmax is
shift-invariant along T — so the decoder branch cancels exactly and both
outputs depend only on softmax_T(tanh(enc @ Wh + bh) @ Wv) and enc itself.
The max-subtraction inside softmax is likewise replaced by a constant
host-computed shift (sum|Wv| + |bh|-slack bounds |score|), so no on-device
max reduction is needed.

Per core (8 batches):
  - big matmul enclinT[u, t] = Wh-tile (stationary) x encT[h, t] (moving) in
    bf16, fp32 PSUM accumulation; tanh (+bh per-partition bias) on ScalarE
  - score[1, t] = Wv-tile (stationary) x tanh tiles, PSUM-accumulated, the
    score matmuls trailing two groups behind the big-MM stream
  - exp on ScalarE with the free-dim accumulator writing the sum into
    column T of the same row; one GpSimdE partition-broadcast of that
    [1, T+1] row feeds both the context multiplies and the 1/sum
  - context in fp32: per h-tile multiply (VectorE) + free-dim reduce
    (VectorE mid-kernel, ScalarE accumulator for the last batch); context
    columns are scaled by 1/sum at the end
Host pre-computes encT (fp32 + bf16) and a j-tiled Wh per core; outputs are
gathered and context^T is rearranged on host.
"""

from contextlib import ExitStack

import numpy as np
import ml_dtypes

import concourse.bass as bass
import concourse.tile as tile
from concourse import bacc, mybir
from concourse.bass_utils import run_bass_kernel_spmd

B, T, H, U = 64, 512, 1024, 1024
NCORES = 8
BL = B // NCORES          # 8 batches per core
P = 128
NH = H // P               # 8 h-tiles
NU = U // P               # 8 u-tiles

f32 = mybir.dt.float32
bf16 = mybir.dt.bfloat16
AF = mybir.ActivationFunctionType
ALU = mybir.AluOpType
BF16 = ml_dtypes.bfloat16


def build_kernel_body(tc, aps, body_ctx):
    nc = tc.nc
    enc_bf_d = aps["enc_bf"]      # [H, BL*T] bf16
    enc_f32_d = aps["enc_f32"]    # [H, BL*T] f32
    wh_d = aps["wh"]              # [NU, H, P] bf16 (j-tiled)
    wv_d = aps["wv"]              # [P, NU] bf16
    bh_d = aps["bh"]              # [P, NU] f32
    nsh_d = aps["nshift"]         # [1, 1] f32: -(bound on score)
    attn_d = aps["attn"]          # [BL, T] f32 out
    ctxt_d = aps["ctxt"]          # [P, NH*BL] f32 out (context^T columns)

    ctx = body_ctx
    cpool = ctx.enter_context(tc.tile_pool(name="const", bufs=1))
    e0_pool = ctx.enter_context(tc.tile_pool(name="e0", bufs=1))
    ebf_pool = ctx.enter_context(tc.tile_pool(name="ebf", bufs=2))
    ef_pool = ctx.enter_context(tc.tile_pool(name="ef", bufs=3))
    th_pool = ctx.enter_context(tc.tile_pool(name="th", bufs=3))
    bc_pool = ctx.enter_context(tc.tile_pool(name="bc", bufs=2))
    scr_pool = ctx.enter_context(tc.tile_pool(name="scr", bufs=3))
    row_pool = ctx.enter_context(tc.tile_pool(name="row", bufs=3))
    ctx_pool = ctx.enter_context(tc.tile_pool(name="ctxt", bufs=1))
    pe_pool = ctx.enter_context(tc.tile_pool(name="pe", bufs=6, space="PSUM"))
    sc_pool = ctx.enter_context(tc.tile_pool(name="sc", bufs=2, space="PSUM"))

    # --- startup-critical loads, all on the sync queue in priority order:
    # j=0 Wh tile, then batch-0 enc bf16 per-h tiles (the first accumulation
    # group can pace with their arrival). Wh j=1..7 go on the scalar queue
    # in per-j tiles so group j only waits for its own 256 KB.
    wh_sb = []
    t_ = cpool.tile([P, H], bf16, tag="wh0")
    nc.sync.dma_start(t_[:], wh_d[0])
    wh_sb.append(t_)

    ebf0 = []
    for h in range(NH):
        t_ = e0_pool.tile([P, T], bf16, tag=f"ebf0_{h}")
        eng = nc.sync if h < 4 else nc.scalar
        eng.dma_start(t_[:], enc_bf_d[0][:, h * T:(h + 1) * T])
        ebf0.append(t_)

    for j in range(1, NU):
        t_ = cpool.tile([P, H], bf16, tag=f"wh{j}")
        eng = nc.sync if j % 2 == 0 else nc.scalar
        eng.dma_start(t_[:], wh_d[j])
        wh_sb.append(t_)

    wv_sb = cpool.tile([P, NU], bf16, tag="wv")
    nc.scalar.dma_start(wv_sb[:], wv_d[:, :])
    bh_sb = cpool.tile([P, NU], f32, tag="bh")
    nc.scalar.dma_start(bh_sb[:], bh_d[:, :])
    nsh_sb = cpool.tile([1, 1], f32, tag="nsh")
    nc.scalar.dma_start(nsh_sb[:], nsh_d[:, :])

    ctx_all = ctx_pool.tile([P, NH * BL], f32, tag="ctxall")
    ctx_view = ctx_all[:].rearrange("p (h b) -> p h b", b=BL)

    # one DMA per batch for the streaming enc tiles
    def load_ebf(b):
        t_ = ebf_pool.tile([P, NH * T], bf16, tag="ebfall")
        nc.sync.dma_start(t_[:], enc_bf_d[b])
        return t_

    def load_ef(b):
        t_ = ef_pool.tile([P, NH * T], f32, tag="efall")
        nc.sync.dma_start(t_[:], enc_f32_d[b])
        return t_

    ef_cur = None
    ebf_cur = None

    def issue_softmax_and_context(b, ef, score_ps, last):
        """Everything after batch b's score matmuls."""
        # exp row with the free-dim sum accumulated into column T
        erow = row_pool.tile([1, T + 1], f32, tag="erow")
        nc.scalar.activation(erow[:, 0:T], score_ps[:], AF.Exp,
                             bias=nsh_sb[:], accum_out=erow[:, T:T + 1])
        # one broadcast feeds the context multiplies AND the 1/sum
        bc = bc_pool.tile([P, T + 1], f32, tag="bc")
        nc.gpsimd.partition_broadcast(bc[:], erow[:])
        rbc = row_pool.tile([P, 1], f32, tag="rbc")
        nc.vector.reciprocal(rbc[:], bc[:, T:T + 1])
        arow = row_pool.tile([1, T], f32, tag="arow")
        nc.vector.tensor_scalar_mul(arow[:], erow[:, 0:T], rbc[0:1, 0:1])
        nc.sync.dma_start(attn_d[b:b + 1, :], arow[:])

        for h in range(NH):
            # one VectorE op per h-tile: (enc * 1/sum) * exp_bc, with the
            # free-dim accumulator writing the normalized context column
            scr = scr_pool.tile([P, T], f32, tag="scr")
            nc.vector.scalar_tensor_tensor(
                out=scr[:], in0=ef[:, h * T:(h + 1) * T], scalar=rbc[:],
                in1=bc[:, 0:T], op0=ALU.mult, op1=ALU.mult,
                accum_out=ctx_view[:, h, b:b + 1])

    def make_score_mm(tanh_ts, score_ps):
        def score_mm(j):
            nc.tensor.matmul(
                score_ps[:], wv_sb[:, j:j + 1], tanh_ts[j][:],
                start=(j == 0), stop=(j == NU - 1), skip_group_check=True,
            )
        return score_mm

    prev = None  # (b-1's tanh tiles, ef tile, score psum)
    for b in range(BL):
        ef = ef_cur
        ebf = ebf_cur
        last = b == BL - 1
        tanh_ts = []
        score_ps = sc_pool.tile([1, T], f32, tag="score")
        score_mm = make_score_mm(tanh_ts, score_ps)
        for j in range(NU):
            pe = pe_pool.tile([P, T], f32, tag="pe")
            for h in range(NH):
                rhs = (ebf0[h][:] if b == 0
                       else ebf[:, h * T:(h + 1) * T])
                nc.tensor.matmul(
                    pe[:], wh_sb[j][:, h * P:(h + 1) * P], rhs,
                    start=(h == 0), stop=(h == NH - 1),
                )
            th = th_pool.tile([P, T], bf16, tag=f"th{j}")
            nc.scalar.activation(th[:], pe[:], AF.Tanh, bias=bh_sb[:, j:j + 1])
            tanh_ts.append(th)
            if j == 0 and prev is not None:
                # previous batch's score matmuls + softmax + context slot in
                # here: tanh(b, 0) is issued first so ScalarE's FIFO reaches
                # it before the exp
                pt, pef, pps = prev
                psc = make_score_mm(pt, pps)
                for jj in range(NU):
                    psc(jj)
                issue_softmax_and_context(b - 1, pef, pps, last=False)
            if last and j >= 2:
                # last batch: scores trail two groups behind so the final
                # block of score matmuls is short
                score_mm(j - 2)
            if j == 1 and b + 1 < BL:
                ebf_cur = load_ebf(b + 1)
                ef_cur = load_ef(b + 1)
            if j == 2 and b == 0:
                ef = load_ef(0)
        if last:
            score_mm(NU - 2)
            score_mm(NU - 1)
            issue_softmax_and_context(b, ef, score_ps, last=True)
        else:
            prev = (tanh_ts, ef, score_ps)

    nc.sync.dma_start(ctxt_d[:, :], ctx_all[:])


def build_nc():
    nc = bacc.Bacc("TRN2", target_bir_lowering=False, debug=False,
                   num_devices=NCORES)
    aps = {
        "enc_bf": nc.dram_tensor("enc_bf", [BL, P, NH * T], bf16,
                                 kind="ExternalInput").ap(),
        "enc_f32": nc.dram_tensor("enc_f32", [BL, P, NH * T], f32,
                                  kind="ExternalInput").ap(),
        "wh": nc.dram_tensor("wh", [NU, P, H], bf16,
                             kind="ExternalInput").ap(),
        "wv": nc.dram_tensor("wv", [P, NU], bf16, kind="ExternalInput").ap(),
        "bh": nc.dram_tensor("bh", [P, NU], f32, kind="ExternalInput").ap(),
        "nshift": nc.dram_tensor("nshift", [1, 1], f32,
                                 kind="ExternalInput").ap(),
        "attn": nc.dram_tensor("attn", [BL, T], f32,
                               kind="ExternalOutput").ap(),
        "ctxt": nc.dram_tensor("ctxt", [P, NH * BL], f32,
                               kind="ExternalOutput").ap(),
    }
    with tile.TileContext(nc) as tc:
        with ExitStack() as body_ctx:
            build_kernel_body(tc, aps, body_ctx)
    nc.compile()
    return nc


def make_in_maps(enc_output, Wh, bh, Wv):
    enc = np.ascontiguousarray(np.asarray(enc_output, dtype=np.float32))
    wh = np.asarray(Wh, dtype=np.float32)
    # SBUF-partition-contiguous Wh: wh_tiled[j, p, h*P+c] = Wh[h*P+p, j*P+c]
    wh_tiled = np.ascontiguousarray(
        wh.reshape(NH, P, NU, P).transpose(2, 1, 0, 3).reshape(NU, P, H)
    ).astype(BF16)
    wv = np.asarray(Wv, dtype=np.float32)
    wv_t = np.ascontiguousarray(wv.reshape(NU, P).T).astype(BF16)
    bh_t = np.ascontiguousarray(
        np.asarray(bh, dtype=np.float32).reshape(NU, P).T)
    # |score| <= sum_u |Wv_u| * |tanh| <= sum|Wv|; constant softmax shift
    # (softmax is shift-invariant, exp(+-bound) stays well inside fp32)
    nshift = np.array([[-min(float(np.abs(wv).sum()), 60.0)]],
                      dtype=np.float32)
    in_maps = []
    for c in range(NCORES):
        shard = enc[c * BL:(c + 1) * BL]              # [BL, T, H]
        # encT_b[b, p, h*T+t] = enc[b, t, h*P+p]
        encT = np.ascontiguousarray(
            shard.reshape(BL, T, NH, P).transpose(0, 3, 2, 1)
        ).reshape(BL, P, NH * T)
        in_maps.append({
            "enc_bf": encT.astype(BF16),
            "enc_f32": encT,
            "wh": wh_tiled,
            "wv": wv_t,
            "bh": bh_t,
            "nshift": nshift,
        })
    return in_maps


_NC_CACHE = None


def kernel(dec_hidden, enc_output, Wh, bh, Ws, bs, Wv, bv, **_unused):
    global _NC_CACHE
    if _NC_CACHE is None:
        _NC_CACHE = build_nc()
    nc = _NC_CACHE
    in_maps = make_in_maps(enc_output, Wh, bh, Wv)
    res = run_bass_kernel_spmd(nc, in_maps, list(range(NCORES))).results
    attn = np.concatenate([res[c]["attn"] for c in range(NCORES)], axis=0)
    ctx_parts = []
    for c in range(NCORES):
        ct = res[c]["ctxt"]                           # [P, NH*BL]
        ct = ct.reshape(P, NH, BL).transpose(2, 1, 0).reshape(BL, H)
        ctx_parts.append(np.ascontiguousarray(ct))
    context = np.concatenate(ctx_parts, axis=0)
    return context.astype(np.float32), attn.astype(np.float32)
